# revision 35
# baseline (speedup 1.0000x reference)
"""BlockNet Trainium2 kernel: data-parallel over 8 NeuronCores.

Design (per core, batch NB=256):
- VALID-conv pyramid trimming: the final [B,4] output only depends on
  x[:, :, 0:47, 0:47], block1 out (i,j)<=14, block2 out <=6, block3 out <=2.
  Everything else is dead and never computed or loaded.
- Host pre-transposes x to xprep[(w,c), h, b] bf16 with constant-1 bias
  rows; all conv biases fold into the matmuls via an extra weight row.
- Each block's untied+shared+gate convs run as ONE fused matmul per
  (jg, i, kh): lhsT [K, 64+S] block-diagonal over j with 32-aligned
  sections [d@0 | s@32 | g@64] (S = nj*cout <= 32), rhs = slab [K, 256]
  at h = s*i+kh, accumulated over kh into a PSUM tile batching G i's.
  A 1-row matmul writes +30 into [96:96+S] so sigmoid gives exact 1.0
  rows ("u" section).
- Epilogue per i-group: gx = sigmoid(acc[64:128]) = [g'; 1] (ACT);
  qs = gx * acc[0:64] = [g'*d ; s] (one PSUM-reading DVE op);
  y = relu(qs_d + qs_s) in cheap all-bf16 DVE ops. All partition bases
  are 32-aligned (HW engine constraint).
- PE warmup matmuls ramp the tensor-engine p-state while DMAs land.
  Slab chunks ride the SP HWDGE queue, weights the Pool SWDGE queue so
  their pre-transfer phases overlap.
- All compute bf16, PSUM accumulation f32.
"""
import numpy as np
import ml_dtypes

import concourse.bass as bass
import concourse.mybir as mybir
import concourse.bacc as bacc
import concourse.tile as tile
from concourse.bass_utils import run_bass_kernel_spmd

N_CORES = 8
NB = 256
BATCH = 2048
BF16 = mybir.dt.bfloat16
F32 = mybir.dt.float32
G = 4       # max i's per PSUM tile
NWARM = 8   # PE warmup matmuls (N=512 each)

# trimmed block configs: (cin, cout, k, s, noh, oh_full)
CFG = [(3, 4, 5, 3, 15, 20), (4, 6, 3, 2, 7, 9), (6, 16, 3, 2, 3, 4),
       (16, 32, 3, 2, 1, 1)]
JGS = [[(0, 8), (8, 7)], [(0, 5), (5, 2)], [(0, 2), (2, 1)], [(0, 1)]]
KS = [[79, 70], [61, 61], [45, 45], [49]]       # rhs partition count per jg
BIASROW = [[78, 69], [60, 60], [44, 44], [48]]  # bias row within the window
# SBUF weight tile shapes [K, noh, k, 64+S]
WSHAPES = {(0, 0): [79, 15, 5, 96], (0, 1): [70, 15, 5, 92],
           (1, 0): [61, 7, 3, 94], (1, 1): [61, 7, 3, 76],
           (2, 0): [45, 3, 3, 96], (2, 1): [45, 3, 3, 80],
           (3, 0): [49, 1, 3, 96]}

_CACHE = {}


def _y2row(w, c):
    """Row of value (w-index, channel) in the padded Y2 layout."""
    return w * 6 + c if w < 5 else 32 + (w - 5) * 6 + c


def _build():
    nc = bacc.Bacc("TRN2", target_bir_lowering=False, debug=False,
                   num_devices=N_CORES)
    xprep = nc.dram_tensor("xprep", [149, 47, NB], BF16, kind="ExternalInput").ap()
    wb = {}
    for key, shp in WSHAPES.items():
        wb[key] = nc.dram_tensor(f"wb{key[0]}_{key[1]}", shp, BF16,
                                 kind="ExternalInput").ap()
    wfc = nc.dram_tensor("wfc", [33, 4], BF16, kind="ExternalInput").ap()
    ones = nc.dram_tensor("ones", [1, 15 * NB], BF16, kind="ExternalInput").ap()
    out_d = nc.dram_tensor("out", [4, NB], F32, kind="ExternalOutput").ap()

    with tile.TileContext(nc) as tc:
        import contextlib
        ctx = contextlib.ExitStack()
        with ctx:
            pconst = ctx.enter_context(tc.tile_pool(name="const", bufs=1))
            pslab = ctx.enter_context(tc.tile_pool(name="slab", bufs=1))
            pg = ctx.enter_context(tc.tile_pool(name="g", bufs=4))
            pq = ctx.enter_context(tc.tile_pool(name="q", bufs=4))
            pp = ctx.enter_context(tc.tile_pool(name="p", bufs=4))
            pc = ctx.enter_context(tc.tile_pool(name="c", bufs=4))
            pps = ctx.enter_context(tc.tile_pool(name="ps", bufs=4, space="PSUM"))

            wfc_t = pconst.tile([33, 4], BF16, tag="wfc")
            out_t = pconst.tile([4, NB], F32, tag="outt")
            scratch = pconst.tile([64, 2, NB], BF16, tag="scratch")

            slab0 = pslab.tile([79, 47, NB], BF16, tag="slab0")
            slab1 = pslab.tile([70, 47, NB], BF16, tag="slab1")
            W = {key: pslab.tile(shp, BF16, tag=f"w{key[0]}_{key[1]}",
                                 name=f"w{key[0]}_{key[1]}")
                 for key, shp in WSHAPES.items()}
            Y1 = pslab.tile([61, 15, NB], BF16, tag="Y1")
            Y2 = pslab.tile([45, 7, NB], BF16, tag="Y2")
            Y3 = pslab.tile([49, 3, NB], BF16, tag="Y3")
            y4 = pslab.tile([33, 1, NB], BF16, tag="y4")
            SRC = [(slab0, slab1), (Y1, Y1), (Y2, Y2), (Y3,)]

            # ---- PE warmup: ramp the tensor engine p-state while DMAs land.
            nc.gpsimd.memset(scratch[:], 0.0)
            for _ in range(NWARM):
                warm = pps.tile([128, G, NB], F32, tag="acc")
                nc.tensor.matmul(warm[:, 0:2, :], scratch[:, 0, 0:128],
                                 scratch[:, 0:2, :], start=True, stop=True)

            # ---- DMA loads: slabs on the SP HWDGE queue, weights on the
            # Pool SWDGE queue; chunks aligned to i-group needs.
            HCH = [(0, 14), (14, 12), (26, 12), (38, 9)]
            ICH = [(0, 4), (4, 4), (8, 4), (12, 3)]
            for t in range(4):
                h0, hn = HCH[t]
                i0, ni = ICH[t]
                nc.sync.dma_start(slab0[:, h0:h0 + hn, :],
                                  xprep[0:79, h0:h0 + hn, :])
                nc.gpsimd.dma_start(W[(0, 0)][:, i0:i0 + ni, :, :],
                                    wb[(0, 0)][:, i0:i0 + ni, :, :])
                nc.sync.dma_start(slab1[:, h0:h0 + hn, :],
                                  xprep[79:149, h0:h0 + hn, :])
                nc.gpsimd.dma_start(W[(0, 1)][:, i0:i0 + ni, :, :],
                                    wb[(0, 1)][:, i0:i0 + ni, :, :])
            for key in [(1, 0), (1, 1), (2, 0), (2, 1), (3, 0)]:
                nc.gpsimd.dma_start(W[key][:], wb[key][:])
            nc.sync.dma_start(wfc_t[:], wfc[:])
            nc.sync.dma_start(Y1[60:61, :, :], ones[:, 0:15 * NB])
            nc.sync.dma_start(Y2[44:45, :, :], ones[:, 0:7 * NB])
            nc.sync.dma_start(Y3[48:49, :, :], ones[:, 0:3 * NB])
            nc.sync.dma_start(y4[32:33, :, :], ones[:, 0:NB])

            ectr = [0]

            def epilogue(acc, S, g, ydst, split=False):
                """acc sections [d@0 | s@32 | g@64] of width S (32-aligned).
                gx = sigmoid(acc_g); q = gx*acc_d (PSUM+SBUF, equal base 0);
                p = q + s; y = relu(p). Block1 (split=True) alternates the
                s-section read between an ACT psum->sbuf copy (then cheap
                all-bf16 DVE add) and a DVE mixed-space psum add, and runs
                relu on Pool, spreading psum reads over three engines."""
                gx = pg.tile([32, G, NB], BF16, tag="g")
                qs = pq.tile([32, G, NB], BF16, tag="q")
                cs = pc.tile([32, G, NB], BF16, tag="c")
                p_t = pp.tile([32, G, NB], BF16, tag="p")
                nc.scalar.activation(gx[0:S, 0:g, :], acc[64:64 + S, 0:g, :],
                                     mybir.ActivationFunctionType.Sigmoid)
                use_act_copy = (not split) or (ectr[0] % 2 == 0)
                ectr[0] += 1
                if use_act_copy:
                    nc.scalar.activation(cs[0:S, 0:g, :], acc[32:32 + S, 0:g, :],
                                         mybir.ActivationFunctionType.Copy)
                nc.vector.tensor_tensor(qs[0:S, 0:g, :], gx[0:S, 0:g, :],
                                        acc[0:S, 0:g, :], mybir.AluOpType.mult)
                if use_act_copy:
                    nc.vector.tensor_tensor(p_t[0:S, 0:g, :], qs[0:S, 0:g, :],
                                            cs[0:S, 0:g, :],
                                            mybir.AluOpType.add)
                else:
                    nc.vector.tensor_tensor(p_t[0:S, 0:g, :], qs[0:S, 0:g, :],
                                            acc[32:32 + S, 0:g, :],
                                            mybir.AluOpType.add)
                if split:
                    nc.gpsimd.tensor_relu(ydst, p_t[0:S, 0:g, :])
                else:
                    nc.vector.tensor_relu(ydst, p_t[0:S, 0:g, :])

            # ---- blocks 1-4 ----
            YOUT = [Y1, Y2, Y3, y4]
            ROW0 = [[0, 32], [0, 32], [0, 32], [0]]
            # tapered i-groups: the last group of each block gates the next
            # block, so keep it small for a short dependency chain.
            IGROUPS = [[(0, 4), (4, 4), (8, 4), (12, 3)],
                       [(0, 2), (2, 2), (4, 2), (6, 1)],
                       [(0, 2), (2, 1)],
                       [(0, 1)]]
            for blk in range(4):
                cin, cout, k, s, noh, ohf = CFG[blk]
                igroups = IGROUPS[blk]
                for i0, gn in igroups:
                    for jg, (j0, nj) in enumerate(JGS[blk]):
                        S = nj * cout
                        K = KS[blk][jg]
                        src = SRC[blk][jg]
                        acc = pps.tile([128, G, NB], F32, tag="acc")
                        for ii in range(gn):
                            i = i0 + ii
                            for kh in range(k):
                                nc.tensor.matmul(
                                    acc[0:64 + S, ii, :],
                                    W[(blk, jg)][0:K, i, kh, :],
                                    src[0:K, s * i + kh, :],
                                    start=(kh == 0), stop=(kh == k - 1))
                        yt = YOUT[blk]
                        r0 = ROW0[blk][jg]
                        if blk == 3:
                            ydst = yt[0:32, 0:1, :]
                        else:
                            ydst = yt[r0:r0 + S, i0:i0 + gn, :]
                        epilogue(acc, S, gn, ydst, split=(blk == 0))

            # ---- FC ----
            accfc = pps.tile([128, G, NB], F32, tag="acc")
            nc.tensor.matmul(accfc[0:4, 0, :], wfc_t[:], y4[0:33, 0, :],
                             start=True, stop=True)
            nc.scalar.activation(out_t[:], accfc[0:4, 0, :],
                                 mybir.ActivationFunctionType.Copy)
            nc.sync.dma_start(out_d[:], out_t[:])

    nc.compile()
    return nc


def _prep_weights(inputs):
    """Fused block-diag weight tensors: 32-aligned sections [d|s|g],
    bias rows folded in (the u rows come from the +30 matmul)."""
    arrs = {}
    for blk in range(4):
        cin, cout, k, s, noh, ohf = CFG[blk]
        wu = np.asarray(inputs[f"w_uc{blk + 1}"], np.float32).reshape(
            ohf * ohf, cin * k * k, cout)
        bu = np.asarray(inputs[f"b_uc{blk + 1}"], np.float32)[0]
        wp = np.asarray(inputs[f"w_pc{blk + 1}"], np.float32)
        bp = np.asarray(inputs[f"b_pc{blk + 1}"], np.float32)
        wg = np.asarray(inputs[f"w_wl{blk + 1}"], np.float32)[0]
        bg = float(np.asarray(inputs[f"b_wl{blk + 1}"], np.float32)[0])

        for jg, (j0, nj) in enumerate(JGS[blk]):
            S = nj * cout
            K = KS[blk][jg]
            brow = BIASROW[blk][jg]
            if blk == 0:
                w0 = 0 if jg == 0 else 24
                rowf = lambda w, c: (w - w0) * 3 + c
            elif blk == 1:
                rowf = lambda w, c: w * 4 + c
            elif blk == 2:
                rowf = lambda w, c: _y2row(w, c)
            else:
                rowf = lambda w, c: w * 16 + c
            Wt = np.zeros((K, noh, k, 64 + S), np.float32)
            ivec = np.arange(noh)
            for jt in range(nj):
                j = j0 + jt
                c0, c1, c2 = jt * cout, 32 + jt * cout, 64 + jt * cout
                for kw in range(k):
                    w = s * j + kw
                    for c in range(cin):
                        row = rowf(w, c)
                        for kh in range(k):
                            un = wu[ivec * ohf + j, c * k * k + kh * k + kw, :]
                            Wt[row, :, kh, c0:c0 + cout] = un - wp[:, c, kh, kw]
                            Wt[row, :, kh, c1:c1 + cout] = wp[:, c, kh, kw]
                            Wt[row, :, kh, c2:c2 + cout] = wg[c, kh, kw]
                Wt[brow, :, 0, c0:c0 + cout] = bu[:, ivec, j].T - bp
                Wt[brow, :, 0, c1:c1 + cout] = bp
                Wt[brow, :, 0, c2:c2 + cout] = bg
            arrs[f"wb{blk}_{jg}"] = Wt.astype(ml_dtypes.bfloat16)

    wfc = np.zeros((33, 4), np.float32)
    wfc[0:32] = np.asarray(inputs["fc_w"], np.float32)
    wfc[32] = np.asarray(inputs["fc_b"], np.float32)
    arrs["wfc"] = wfc.astype(ml_dtypes.bfloat16)
    arrs["ones"] = np.ones((1, 15 * NB), ml_dtypes.bfloat16)
    return arrs


def make_in_maps(inputs):
    warrs = _prep_weights(inputs)
    x = np.asarray(inputs["x"], np.float32)
    # [w, c, h, b] view of the live x region, rows (w*3+c)
    xt = np.ascontiguousarray(x[:, :, 0:47, 0:47].transpose(3, 1, 2, 0))
    xt = xt.reshape(141, 47, BATCH).astype(ml_dtypes.bfloat16)
    in_maps = []
    for ci in range(N_CORES):
        xc = xt[:, :, ci * NB:(ci + 1) * NB]
        xprep = np.empty((149, 47, NB), ml_dtypes.bfloat16)
        xprep[0:78] = xc[0:78]          # slab0: w 0..25
        xprep[78] = 1.0
        xprep[79:148] = xc[72:141]      # slab1: w 24..46
        xprep[148] = 1.0
        m = {"xprep": xprep}
        m.update(warrs)
        in_maps.append(m)
    return in_maps


def kernel(**inputs):
    if "nc" not in _CACHE:
        _CACHE["nc"] = _build()
    nc = _CACHE["nc"]
    in_maps = make_in_maps(inputs)
    res = run_bass_kernel_spmd(nc, in_maps, core_ids=list(range(N_CORES)))
    out = np.concatenate([res.results[c]["out"].T for c in range(N_CORES)], axis=0)
    return out.astype(np.float32)


# revision 41
# speedup vs baseline: 1.0091x; 1.0091x over previous
"""BlockNet Trainium2 kernel: data-parallel over 8 NeuronCores.

Design (per core, batch NB=256):
- VALID-conv pyramid trimming: the final [B,4] output only depends on
  x[:, :, 0:47, 0:47], block1 out (i,j)<=14, block2 out <=6, block3 out <=2.
  Everything else is dead and never computed or loaded.
- Host pre-transposes x to xprep[(w,c), h, b] bf16 with constant-1 bias
  rows; all conv biases fold into the matmuls via an extra weight row.
- Each block's untied+shared+gate convs run as ONE fused matmul per
  (jg, i, kh): lhsT [K, 64+S] block-diagonal over j with 32-aligned
  sections [d@0 | s@32 | g@64] (S = nj*cout <= 32), rhs = slab [K, 256]
  at h = s*i+kh, accumulated over kh into a PSUM tile batching G i's.
  A 1-row matmul writes +30 into [96:96+S] so sigmoid gives exact 1.0
  rows ("u" section).
- Epilogue per i-group: gx = sigmoid(acc[64:128]) = [g'; 1] (ACT);
  qs = gx * acc[0:64] = [g'*d ; s] (one PSUM-reading DVE op);
  y = relu(qs_d + qs_s) in cheap all-bf16 DVE ops. All partition bases
  are 32-aligned (HW engine constraint).
- PE warmup matmuls ramp the tensor-engine p-state while DMAs land.
  Slab chunks ride the SP HWDGE queue, weights the Pool SWDGE queue so
  their pre-transfer phases overlap.
- All compute bf16, PSUM accumulation f32.
"""
import numpy as np
import ml_dtypes

import concourse.bass as bass
import concourse.mybir as mybir
import concourse.bacc as bacc
import concourse.tile as tile
from concourse.bass_utils import run_bass_kernel_spmd

N_CORES = 8
NB = 256
BATCH = 2048
BF16 = mybir.dt.bfloat16
F32 = mybir.dt.float32
G = 4       # max i's per PSUM tile
NWARM = 8   # PE warmup matmuls (N=512 each)

# trimmed block configs: (cin, cout, k, s, noh, oh_full)
CFG = [(3, 4, 5, 3, 15, 20), (4, 6, 3, 2, 7, 9), (6, 16, 3, 2, 3, 4),
       (16, 32, 3, 2, 1, 1)]
JGS = [[(0, 8), (8, 7)], [(0, 5), (5, 2)], [(0, 2), (2, 1)], [(0, 1)]]
KS = [[79, 70], [61, 61], [45, 45], [49]]       # rhs partition count per jg
BIASROW = [[78, 69], [60, 60], [44, 44], [48]]  # bias row within the window
# SBUF weight tile shapes [K, noh, k, 64+S]
WSHAPES = {(0, 0): [79, 15, 5, 96], (0, 1): [70, 15, 5, 92],
           (1, 0): [61, 7, 3, 94], (1, 1): [61, 7, 3, 76],
           (2, 0): [45, 3, 3, 96], (2, 1): [45, 3, 3, 80],
           (3, 0): [49, 1, 3, 96]}

_CACHE = {}


def _y2row(w, c):
    """Row of value (w-index, channel) in the padded Y2 layout."""
    return w * 6 + c if w < 5 else 32 + (w - 5) * 6 + c


def _build():
    nc = bacc.Bacc("TRN2", target_bir_lowering=False, debug=False,
                   num_devices=N_CORES)
    xprep = nc.dram_tensor("xprep", [149, 47, NB], BF16, kind="ExternalInput").ap()
    wb = {}
    for key, shp in WSHAPES.items():
        wb[key] = nc.dram_tensor(f"wb{key[0]}_{key[1]}", shp, BF16,
                                 kind="ExternalInput").ap()
    wfc = nc.dram_tensor("wfc", [33, 4], BF16, kind="ExternalInput").ap()
    ones = nc.dram_tensor("ones", [1, 15 * NB], BF16, kind="ExternalInput").ap()
    out_d = nc.dram_tensor("out", [4, NB], F32, kind="ExternalOutput").ap()

    with tile.TileContext(nc) as tc:
        import contextlib
        ctx = contextlib.ExitStack()
        with ctx:
            pconst = ctx.enter_context(tc.tile_pool(name="const", bufs=1))
            pslab = ctx.enter_context(tc.tile_pool(name="slab", bufs=1))
            pg = ctx.enter_context(tc.tile_pool(name="g", bufs=4))
            pq = ctx.enter_context(tc.tile_pool(name="q", bufs=4))
            pp = ctx.enter_context(tc.tile_pool(name="p", bufs=4))
            pc = ctx.enter_context(tc.tile_pool(name="c", bufs=4))
            pps = ctx.enter_context(tc.tile_pool(name="ps", bufs=4, space="PSUM"))

            wfc_t = pconst.tile([33, 4], BF16, tag="wfc")
            out_t = pconst.tile([4, NB], F32, tag="outt")
            scratch = pconst.tile([64, 2, NB], BF16, tag="scratch")

            slab0 = pslab.tile([79, 47, NB], BF16, tag="slab0")
            slab1 = pslab.tile([70, 47, NB], BF16, tag="slab1")
            W = {key: pslab.tile(shp, BF16, tag=f"w{key[0]}_{key[1]}",
                                 name=f"w{key[0]}_{key[1]}")
                 for key, shp in WSHAPES.items()}
            Y1 = pslab.tile([61, 15, NB], BF16, tag="Y1")
            Y2 = pslab.tile([45, 7, NB], BF16, tag="Y2")
            Y3 = pslab.tile([49, 3, NB], BF16, tag="Y3")
            y4 = pslab.tile([33, 1, NB], BF16, tag="y4")
            SRC = [(slab0, slab1), (Y1, Y1), (Y2, Y2), (Y3,)]

            # ---- PE warmup: ramp the tensor engine p-state while DMAs land.
            nc.gpsimd.memset(scratch[:], 0.0)
            for _ in range(NWARM):
                warm = pps.tile([128, G, NB], F32, tag="acc")
                nc.tensor.matmul(warm[:, 0:2, :], scratch[:, 0, 0:128],
                                 scratch[:, 0:2, :], start=True, stop=True)

            # ---- DMA loads: slabs on the SP HWDGE queue, weights on the
            # Pool SWDGE queue; chunks aligned to i-group needs.
            HCH = [(0, 14), (14, 12), (26, 12), (38, 9)]
            ICH = [(0, 4), (4, 4), (8, 4), (12, 3)]
            for t in range(4):
                h0, hn = HCH[t]
                i0, ni = ICH[t]
                nc.sync.dma_start(slab0[:, h0:h0 + hn, :],
                                  xprep[0:79, h0:h0 + hn, :])
                nc.gpsimd.dma_start(W[(0, 0)][:, i0:i0 + ni, :, :],
                                    wb[(0, 0)][:, i0:i0 + ni, :, :])
                nc.sync.dma_start(slab1[:, h0:h0 + hn, :],
                                  xprep[79:149, h0:h0 + hn, :])
                nc.gpsimd.dma_start(W[(0, 1)][:, i0:i0 + ni, :, :],
                                    wb[(0, 1)][:, i0:i0 + ni, :, :])
            for key in [(1, 0), (1, 1), (2, 0), (2, 1), (3, 0)]:
                nc.gpsimd.dma_start(W[key][:], wb[key][:])
            nc.sync.dma_start(wfc_t[:], wfc[:])
            nc.sync.dma_start(Y1[60:61, :, :], ones[:, 0:15 * NB])
            nc.sync.dma_start(Y2[44:45, :, :], ones[:, 0:7 * NB])
            nc.sync.dma_start(Y3[48:49, :, :], ones[:, 0:3 * NB])
            nc.sync.dma_start(y4[32:33, :, :], ones[:, 0:NB])

            ectr = [0]

            def epilogue(acc, S, g, ydst, split=False):
                """acc sections [d@0 | s@32 | g@64] of width S (32-aligned).
                gx = sigmoid(acc_g); q = gx*acc_d (PSUM+SBUF, equal base 0);
                p = q + s; y = relu(p). Block1 (split=True) alternates the
                s-section read between an ACT psum->sbuf copy (then cheap
                all-bf16 DVE add) and a DVE mixed-space psum add, and runs
                relu on Pool, spreading psum reads over three engines."""
                gx = pg.tile([32, G, NB], BF16, tag="g")
                qs = pq.tile([32, G, NB], BF16, tag="q")
                cs = pc.tile([32, G, NB], BF16, tag="c")
                p_t = pp.tile([32, G, NB], BF16, tag="p")
                nc.scalar.activation(gx[0:S, 0:g, :], acc[64:64 + S, 0:g, :],
                                     mybir.ActivationFunctionType.Sigmoid)
                use_act_copy = (not split) or (ectr[0] % 2 == 0)
                ectr[0] += 1
                if use_act_copy:
                    nc.scalar.activation(cs[0:S, 0:g, :], acc[32:32 + S, 0:g, :],
                                         mybir.ActivationFunctionType.Copy)
                nc.vector.tensor_tensor(qs[0:S, 0:g, :], gx[0:S, 0:g, :],
                                        acc[0:S, 0:g, :], mybir.AluOpType.mult)
                if use_act_copy:
                    nc.vector.tensor_tensor(p_t[0:S, 0:g, :], qs[0:S, 0:g, :],
                                            cs[0:S, 0:g, :],
                                            mybir.AluOpType.add)
                else:
                    nc.vector.tensor_tensor(p_t[0:S, 0:g, :], qs[0:S, 0:g, :],
                                            acc[32:32 + S, 0:g, :],
                                            mybir.AluOpType.add)
                if split:
                    nc.gpsimd.tensor_relu(ydst, p_t[0:S, 0:g, :])
                else:
                    nc.vector.tensor_relu(ydst, p_t[0:S, 0:g, :])

            # ---- blocks 1-4 ----
            YOUT = [Y1, Y2, Y3, y4]
            ROW0 = [[0, 32], [0, 32], [0, 32], [0]]
            # tapered i-groups: the last group of each block gates the next
            # block, so keep it small for a short dependency chain.
            IGROUPS = [[(0, 4), (4, 4), (8, 4), (12, 3)],
                       [(0, 2), (2, 2), (4, 2), (6, 1)],
                       [(0, 1), (1, 1), (2, 1)],
                       [(0, 1)]]
            for blk in range(4):
                cin, cout, k, s, noh, ohf = CFG[blk]
                igroups = IGROUPS[blk]
                for i0, gn in igroups:
                    for jg, (j0, nj) in enumerate(JGS[blk]):
                        S = nj * cout
                        K = KS[blk][jg]
                        src = SRC[blk][jg]
                        acc = pps.tile([128, G, NB], F32, tag="acc")
                        for ii in range(gn):
                            i = i0 + ii
                            for kh in range(k):
                                nc.tensor.matmul(
                                    acc[0:64 + S, ii, :],
                                    W[(blk, jg)][0:K, i, kh, :],
                                    src[0:K, s * i + kh, :],
                                    start=(kh == 0), stop=(kh == k - 1))
                        yt = YOUT[blk]
                        r0 = ROW0[blk][jg]
                        if blk == 3:
                            ydst = yt[0:32, 0:1, :]
                        else:
                            ydst = yt[r0:r0 + S, i0:i0 + gn, :]
                        epilogue(acc, S, gn, ydst, split=(blk == 0))

            # ---- FC ----
            accfc = pps.tile([128, G, NB], F32, tag="acc")
            nc.tensor.matmul(accfc[0:4, 0, :], wfc_t[:], y4[0:33, 0, :],
                             start=True, stop=True)
            nc.scalar.activation(out_t[:], accfc[0:4, 0, :],
                                 mybir.ActivationFunctionType.Copy)
            nc.sync.dma_start(out_d[:], out_t[:])

    nc.compile()
    return nc


def _prep_weights(inputs):
    """Fused block-diag weight tensors: 32-aligned sections [d|s|g],
    bias rows folded in (the u rows come from the +30 matmul)."""
    arrs = {}
    for blk in range(4):
        cin, cout, k, s, noh, ohf = CFG[blk]
        wu = np.asarray(inputs[f"w_uc{blk + 1}"], np.float32).reshape(
            ohf * ohf, cin * k * k, cout)
        bu = np.asarray(inputs[f"b_uc{blk + 1}"], np.float32)[0]
        wp = np.asarray(inputs[f"w_pc{blk + 1}"], np.float32)
        bp = np.asarray(inputs[f"b_pc{blk + 1}"], np.float32)
        wg = np.asarray(inputs[f"w_wl{blk + 1}"], np.float32)[0]
        bg = float(np.asarray(inputs[f"b_wl{blk + 1}"], np.float32)[0])

        for jg, (j0, nj) in enumerate(JGS[blk]):
            S = nj * cout
            K = KS[blk][jg]
            brow = BIASROW[blk][jg]
            if blk == 0:
                w0 = 0 if jg == 0 else 24
                rowf = lambda w, c: (w - w0) * 3 + c
            elif blk == 1:
                rowf = lambda w, c: w * 4 + c
            elif blk == 2:
                rowf = lambda w, c: _y2row(w, c)
            else:
                rowf = lambda w, c: w * 16 + c
            Wt = np.zeros((K, noh, k, 64 + S), np.float32)
            ivec = np.arange(noh)
            for jt in range(nj):
                j = j0 + jt
                c0, c1, c2 = jt * cout, 32 + jt * cout, 64 + jt * cout
                for kw in range(k):
                    w = s * j + kw
                    for c in range(cin):
                        row = rowf(w, c)
                        for kh in range(k):
                            un = wu[ivec * ohf + j, c * k * k + kh * k + kw, :]
                            Wt[row, :, kh, c0:c0 + cout] = un - wp[:, c, kh, kw]
                            Wt[row, :, kh, c1:c1 + cout] = wp[:, c, kh, kw]
                            Wt[row, :, kh, c2:c2 + cout] = wg[c, kh, kw]
                Wt[brow, :, 0, c0:c0 + cout] = bu[:, ivec, j].T - bp
                Wt[brow, :, 0, c1:c1 + cout] = bp
                Wt[brow, :, 0, c2:c2 + cout] = bg
            arrs[f"wb{blk}_{jg}"] = Wt.astype(ml_dtypes.bfloat16)

    wfc = np.zeros((33, 4), np.float32)
    wfc[0:32] = np.asarray(inputs["fc_w"], np.float32)
    wfc[32] = np.asarray(inputs["fc_b"], np.float32)
    arrs["wfc"] = wfc.astype(ml_dtypes.bfloat16)
    arrs["ones"] = np.ones((1, 15 * NB), ml_dtypes.bfloat16)
    return arrs


def make_in_maps(inputs):
    warrs = _prep_weights(inputs)
    x = np.asarray(inputs["x"], np.float32)
    # [w, c, h, b] view of the live x region, rows (w*3+c)
    xt = np.ascontiguousarray(x[:, :, 0:47, 0:47].transpose(3, 1, 2, 0))
    xt = xt.reshape(141, 47, BATCH).astype(ml_dtypes.bfloat16)
    in_maps = []
    for ci in range(N_CORES):
        xc = xt[:, :, ci * NB:(ci + 1) * NB]
        xprep = np.empty((149, 47, NB), ml_dtypes.bfloat16)
        xprep[0:78] = xc[0:78]          # slab0: w 0..25
        xprep[78] = 1.0
        xprep[79:148] = xc[72:141]      # slab1: w 24..46
        xprep[148] = 1.0
        m = {"xprep": xprep}
        m.update(warrs)
        in_maps.append(m)
    return in_maps


def kernel(**inputs):
    if "nc" not in _CACHE:
        _CACHE["nc"] = _build()
    nc = _CACHE["nc"]
    in_maps = make_in_maps(inputs)
    res = run_bass_kernel_spmd(nc, in_maps, core_ids=list(range(N_CORES)))
    out = np.concatenate([res.results[c]["out"].T for c in range(N_CORES)], axis=0)
    return out.astype(np.float32)


# revision 52
# speedup vs baseline: 1.0799x; 1.0701x over previous
"""BlockNet Trainium2 kernel: data-parallel over 8 NeuronCores.

Design (per core, batch NB=256):
- VALID-conv pyramid trimming: the final [B,4] output only depends on
  x[:, :, 0:47, 0:47], block1 out (i,j)<=14, block2 out <=6, block3 out <=2.
  Everything else is dead and never computed or loaded.
- Host pre-transposes x to xprep[(w,c), h, b] bf16 with constant-1 bias
  rows; all conv biases fold into the matmuls via an extra weight row.
- Each block's untied+shared+gate convs run as ONE fused matmul per
  (jg, i, kh): lhsT [K, 64+S] block-diagonal over j with 32-aligned
  sections [d@0 | s@32 | g@64] (S = nj*cout <= 32), rhs = slab [K, 256]
  at h = s*i+kh, accumulated over kh into a PSUM tile batching G i's.
  A 1-row matmul writes +30 into [96:96+S] so sigmoid gives exact 1.0
  rows ("u" section).
- Epilogue per i-group: gx = sigmoid(acc[64:128]) = [g'; 1] (ACT);
  qs = gx * acc[0:64] = [g'*d ; s] (one PSUM-reading DVE op);
  y = relu(qs_d + qs_s) in cheap all-bf16 DVE ops. All partition bases
  are 32-aligned (HW engine constraint).
- PE warmup matmuls ramp the tensor-engine p-state while DMAs land.
  Slab chunks ride the SP HWDGE queue, weights the Pool SWDGE queue so
  their pre-transfer phases overlap.
- All compute bf16, PSUM accumulation f32.
"""
import numpy as np
import ml_dtypes

import concourse.bass as bass
import concourse.mybir as mybir
import concourse.bacc as bacc
import concourse.tile as tile
from concourse.bass_utils import run_bass_kernel_spmd

N_CORES = 8
NB = 256
BATCH = 2048
BF16 = mybir.dt.bfloat16
F32 = mybir.dt.float32
G = 4       # max i's per PSUM tile
NWARM = 5   # PE warmup matmuls (N=512 each)

# trimmed block configs: (cin, cout, k, s, noh, oh_full)
CFG = [(3, 4, 5, 3, 15, 20), (4, 6, 3, 2, 7, 9), (6, 16, 3, 2, 3, 4),
       (16, 32, 3, 2, 1, 1)]
JGS = [[(0, 8), (8, 7)], [(0, 5), (5, 2)], [(0, 2), (2, 1)], [(0, 1)]]
KS = [[79, 70], [61, 61], [45, 45], [49]]       # rhs partition count per jg
BIASROW = [[78, 69], [60, 60], [44, 44], [48]]  # bias row within the window
# SBUF weight tile shapes [K, noh, k, 64+S]
WSHAPES = {(0, 0): [79, 15, 5, 96], (0, 1): [70, 15, 5, 92],
           (1, 0): [61, 7, 3, 94], (1, 1): [61, 7, 3, 76],
           (2, 0): [45, 3, 3, 96], (2, 1): [45, 3, 3, 80],
           (3, 0): [49, 1, 3, 96]}

_CACHE = {}


def _y2row(w, c):
    """Row of value (w-index, channel) in the padded Y2 layout."""
    return w * 6 + c if w < 5 else 32 + (w - 5) * 6 + c


def _build():
    nc = bacc.Bacc("TRN2", target_bir_lowering=False, debug=False,
                   num_devices=N_CORES)
    xprep = nc.dram_tensor("xprep", [149, 47, NB], BF16, kind="ExternalInput").ap()
    wb = {}
    for key, shp in WSHAPES.items():
        wb[key] = nc.dram_tensor(f"wb{key[0]}_{key[1]}", shp, BF16,
                                 kind="ExternalInput").ap()
    wfc = nc.dram_tensor("wfc", [33, 4], BF16, kind="ExternalInput").ap()
    ones = nc.dram_tensor("ones", [1, 15 * NB], BF16, kind="ExternalInput").ap()
    out_d = nc.dram_tensor("out", [4, NB], F32, kind="ExternalOutput").ap()

    with tile.TileContext(nc) as tc:
        import contextlib
        ctx = contextlib.ExitStack()
        with ctx:
            pconst = ctx.enter_context(tc.tile_pool(name="const", bufs=1))
            pslab = ctx.enter_context(tc.tile_pool(name="slab", bufs=1))
            pg = ctx.enter_context(tc.tile_pool(name="g", bufs=4))
            pq = ctx.enter_context(tc.tile_pool(name="q", bufs=4))
            pp = ctx.enter_context(tc.tile_pool(name="p", bufs=4))
            pc = ctx.enter_context(tc.tile_pool(name="c", bufs=4))
            pps = ctx.enter_context(tc.tile_pool(name="ps", bufs=4, space="PSUM"))

            wfc_t = pconst.tile([33, 4], BF16, tag="wfc")
            out_t = pconst.tile([4, NB], F32, tag="outt")
            scratch = pconst.tile([64, 2, NB], BF16, tag="scratch")

            slab0 = pslab.tile([79, 47, NB], BF16, tag="slab0")
            slab1 = pslab.tile([70, 47, NB], BF16, tag="slab1")
            W = {key: pslab.tile(shp, BF16, tag=f"w{key[0]}_{key[1]}",
                                 name=f"w{key[0]}_{key[1]}")
                 for key, shp in WSHAPES.items()}
            Y1 = pslab.tile([61, 15, NB], BF16, tag="Y1")
            Y2 = pslab.tile([45, 7, NB], BF16, tag="Y2")
            Y3 = pslab.tile([49, 3, NB], BF16, tag="Y3")
            y4 = pslab.tile([33, 1, NB], BF16, tag="y4")
            SRC = [(slab0, slab1), (Y1, Y1), (Y2, Y2), (Y3,)]

            # ---- PE warmup: ramp the tensor engine p-state while DMAs land.
            nc.gpsimd.memset(scratch[:], 0.0)
            for _ in range(NWARM):
                warm = pps.tile([128, G, NB], F32, tag="acc")
                nc.tensor.matmul(warm[:, 0:2, :], scratch[:, 0, 0:128],
                                 scratch[:, 0:2, :], start=True, stop=True)

            # ---- DMA loads: slabs on the SP HWDGE queue, weights on the
            # Pool SWDGE queue; chunks aligned to i-group needs.
            HCH = [(0, 8), (8, 6), (14, 12), (26, 12), (38, 9)]
            ICH = [(0, 4), (4, 0), (4, 4), (8, 4), (12, 3)]
            # first weight chunks ride the SP queue ahead of the slabs so the
            # first matmul group's deps land earliest
            nc.sync.dma_start(W[(0, 0)][:, 0:4, :, :], wb[(0, 0)][:, 0:4, :, :])
            for t in range(5):
                h0, hn = HCH[t]
                i0, ni = ICH[t]
                nc.sync.dma_start(slab0[:, h0:h0 + hn, :],
                                  xprep[0:79, h0:h0 + hn, :])
                if t > 0 and ni:
                    nc.gpsimd.dma_start(W[(0, 0)][:, i0:i0 + ni, :, :],
                                        wb[(0, 0)][:, i0:i0 + ni, :, :])
                nc.sync.dma_start(slab1[:, h0:h0 + hn, :],
                                  xprep[79:149, h0:h0 + hn, :])
                if ni:
                    nc.gpsimd.dma_start(W[(0, 1)][:, i0:i0 + ni, :, :],
                                        wb[(0, 1)][:, i0:i0 + ni, :, :])
            for key in [(1, 0), (1, 1), (2, 0), (2, 1), (3, 0)]:
                nc.gpsimd.dma_start(W[key][:], wb[key][:])
            nc.sync.dma_start(wfc_t[:], wfc[:])
            nc.sync.dma_start(Y1[60:61, :, :], ones[:, 0:15 * NB])
            nc.sync.dma_start(Y2[44:45, :, :], ones[:, 0:7 * NB])
            nc.sync.dma_start(Y3[48:49, :, :], ones[:, 0:3 * NB])
            nc.sync.dma_start(y4[32:33, :, :], ones[:, 0:NB])

            ectr = [0]

            def epilogue(acc, S, g, ydst, split=False):
                """acc sections [d@0 | s@32 | g@64] of width S (32-aligned).
                gx = sigmoid(acc_g); q = gx*acc_d (PSUM+SBUF, equal base 0);
                p = q + s; y = relu(p). Block1 (split=True) alternates the
                s-section read between an ACT psum->sbuf copy (then cheap
                all-bf16 DVE add) and a DVE mixed-space psum add, and runs
                relu on Pool, spreading psum reads over three engines."""
                gx = pg.tile([32, G, NB], BF16, tag="g")
                qs = pq.tile([32, G, NB], BF16, tag="q")
                cs = pc.tile([32, G, NB], BF16, tag="c")
                p_t = pp.tile([32, G, NB], BF16, tag="p")
                nc.scalar.activation(gx[0:S, 0:g, :], acc[64:64 + S, 0:g, :],
                                     mybir.ActivationFunctionType.Sigmoid)
                use_act_copy = (not split) or (ectr[0] % 2 == 0)
                ectr[0] += 1
                if use_act_copy:
                    nc.scalar.activation(cs[0:S, 0:g, :], acc[32:32 + S, 0:g, :],
                                         mybir.ActivationFunctionType.Copy)
                nc.vector.tensor_tensor(qs[0:S, 0:g, :], gx[0:S, 0:g, :],
                                        acc[0:S, 0:g, :], mybir.AluOpType.mult)
                if use_act_copy:
                    nc.vector.tensor_tensor(p_t[0:S, 0:g, :], qs[0:S, 0:g, :],
                                            cs[0:S, 0:g, :],
                                            mybir.AluOpType.add)
                else:
                    nc.vector.tensor_tensor(p_t[0:S, 0:g, :], qs[0:S, 0:g, :],
                                            acc[32:32 + S, 0:g, :],
                                            mybir.AluOpType.add)
                if split:
                    nc.gpsimd.tensor_relu(ydst, p_t[0:S, 0:g, :])
                else:
                    nc.vector.tensor_relu(ydst, p_t[0:S, 0:g, :])

            # ---- blocks 1-4 ----
            def pe_filler(n):
                for _ in range(n):
                    warm = pps.tile([128, G, NB], F32, tag="acc")
                    nc.tensor.matmul(warm[:, 0:2, :], scratch[:, 0, 0:128],
                                     scratch[:, 0:2, :], start=True, stop=True)

            YOUT = [Y1, Y2, Y3, y4]
            ROW0 = [[0, 32], [0, 32], [0, 32], [0]]
            # tapered i-groups: the last group of each block gates the next
            # block, so keep it small for a short dependency chain.
            IGROUPS = [[(0, 4), (4, 4), (8, 4), (12, 3)],
                       [(0, 2), (2, 2), (4, 2), (6, 1)],
                       [(0, 1), (1, 1), (2, 1)],
                       [(0, 1)]]
            for blk in range(4):
                if blk >= 1:
                    pe_filler(2)
                cin, cout, k, s, noh, ohf = CFG[blk]
                igroups = IGROUPS[blk]
                for i0, gn in igroups:
                    for jg, (j0, nj) in enumerate(JGS[blk]):
                        S = nj * cout
                        K = KS[blk][jg]
                        src = SRC[blk][jg]
                        acc = pps.tile([128, G, NB], F32, tag="acc")
                        for ii in range(gn):
                            i = i0 + ii
                            for kh in range(k):
                                nc.tensor.matmul(
                                    acc[0:64 + S, ii, :],
                                    W[(blk, jg)][0:K, i, kh, :],
                                    src[0:K, s * i + kh, :],
                                    start=(kh == 0), stop=(kh == k - 1))
                        yt = YOUT[blk]
                        r0 = ROW0[blk][jg]
                        if blk == 3:
                            ydst = yt[0:32, 0:1, :]
                        else:
                            ydst = yt[r0:r0 + S, i0:i0 + gn, :]
                        epilogue(acc, S, gn, ydst, split=(blk == 0))

            # ---- FC ----
            accfc = pps.tile([128, G, NB], F32, tag="acc")
            nc.tensor.matmul(accfc[0:4, 0, :], wfc_t[:], y4[0:33, 0, :],
                             start=True, stop=True)
            nc.scalar.activation(out_t[:], accfc[0:4, 0, :],
                                 mybir.ActivationFunctionType.Copy)
            nc.sync.dma_start(out_d[:], out_t[:])

    nc.compile()
    return nc


def _prep_weights(inputs):
    """Fused block-diag weight tensors: 32-aligned sections [d|s|g],
    bias rows folded in (the u rows come from the +30 matmul)."""
    arrs = {}
    for blk in range(4):
        cin, cout, k, s, noh, ohf = CFG[blk]
        wu = np.asarray(inputs[f"w_uc{blk + 1}"], np.float32).reshape(
            ohf * ohf, cin * k * k, cout)
        bu = np.asarray(inputs[f"b_uc{blk + 1}"], np.float32)[0]
        wp = np.asarray(inputs[f"w_pc{blk + 1}"], np.float32)
        bp = np.asarray(inputs[f"b_pc{blk + 1}"], np.float32)
        wg = np.asarray(inputs[f"w_wl{blk + 1}"], np.float32)[0]
        bg = float(np.asarray(inputs[f"b_wl{blk + 1}"], np.float32)[0])

        for jg, (j0, nj) in enumerate(JGS[blk]):
            S = nj * cout
            K = KS[blk][jg]
            brow = BIASROW[blk][jg]
            if blk == 0:
                w0 = 0 if jg == 0 else 24
                rowf = lambda w, c: (w - w0) * 3 + c
            elif blk == 1:
                rowf = lambda w, c: w * 4 + c
            elif blk == 2:
                rowf = lambda w, c: _y2row(w, c)
            else:
                rowf = lambda w, c: w * 16 + c
            Wt = np.zeros((K, noh, k, 64 + S), np.float32)
            ivec = np.arange(noh)
            for jt in range(nj):
                j = j0 + jt
                c0, c1, c2 = jt * cout, 32 + jt * cout, 64 + jt * cout
                for kw in range(k):
                    w = s * j + kw
                    for c in range(cin):
                        row = rowf(w, c)
                        for kh in range(k):
                            un = wu[ivec * ohf + j, c * k * k + kh * k + kw, :]
                            Wt[row, :, kh, c0:c0 + cout] = un - wp[:, c, kh, kw]
                            Wt[row, :, kh, c1:c1 + cout] = wp[:, c, kh, kw]
                            Wt[row, :, kh, c2:c2 + cout] = wg[c, kh, kw]
                Wt[brow, :, 0, c0:c0 + cout] = bu[:, ivec, j].T - bp
                Wt[brow, :, 0, c1:c1 + cout] = bp
                Wt[brow, :, 0, c2:c2 + cout] = bg
            arrs[f"wb{blk}_{jg}"] = Wt.astype(ml_dtypes.bfloat16)

    wfc = np.zeros((33, 4), np.float32)
    wfc[0:32] = np.asarray(inputs["fc_w"], np.float32)
    wfc[32] = np.asarray(inputs["fc_b"], np.float32)
    arrs["wfc"] = wfc.astype(ml_dtypes.bfloat16)
    arrs["ones"] = np.ones((1, 15 * NB), ml_dtypes.bfloat16)
    return arrs


def make_in_maps(inputs):
    warrs = _prep_weights(inputs)
    x = np.asarray(inputs["x"], np.float32)
    # [w, c, h, b] view of the live x region, rows (w*3+c)
    xt = np.ascontiguousarray(x[:, :, 0:47, 0:47].transpose(3, 1, 2, 0))
    xt = xt.reshape(141, 47, BATCH).astype(ml_dtypes.bfloat16)
    in_maps = []
    for ci in range(N_CORES):
        xc = xt[:, :, ci * NB:(ci + 1) * NB]
        xprep = np.empty((149, 47, NB), ml_dtypes.bfloat16)
        xprep[0:78] = xc[0:78]          # slab0: w 0..25
        xprep[78] = 1.0
        xprep[79:148] = xc[72:141]      # slab1: w 24..46
        xprep[148] = 1.0
        m = {"xprep": xprep}
        m.update(warrs)
        in_maps.append(m)
    return in_maps


def kernel(**inputs):
    if "nc" not in _CACHE:
        _CACHE["nc"] = _build()
    nc = _CACHE["nc"]
    in_maps = make_in_maps(inputs)
    res = run_bass_kernel_spmd(nc, in_maps, core_ids=list(range(N_CORES)))
    out = np.concatenate([res.results[c]["out"].T for c in range(N_CORES)], axis=0)
    return out.astype(np.float32)


# revision 63
# speedup vs baseline: 1.0845x; 1.0042x over previous
"""BlockNet Trainium2 kernel: data-parallel over 8 NeuronCores.

Design (per core, batch NB=256):
- VALID-conv pyramid trimming: the final [B,4] output only depends on
  x[:, :, 0:47, 0:47], block1 out (i,j)<=14, block2 out <=6, block3 out <=2.
  Everything else is dead and never computed or loaded.
- Host pre-transposes x to xprep[(w,c), h, b] bf16 with constant-1 bias
  rows; all conv biases fold into the matmuls via an extra weight row.
- Each block's untied+shared+gate convs run as ONE fused matmul per
  (jg, i, kh): lhsT [K, 64+S] block-diagonal over j with 32-aligned
  sections [d@0 | s@32 | g@64] (S = nj*cout <= 32), rhs = slab [K, 256]
  at h = s*i+kh, accumulated over kh into a PSUM tile batching G i's.
  A 1-row matmul writes +30 into [96:96+S] so sigmoid gives exact 1.0
  rows ("u" section).
- Epilogue per i-group: gx = sigmoid(acc[64:128]) = [g'; 1] (ACT);
  qs = gx * acc[0:64] = [g'*d ; s] (one PSUM-reading DVE op);
  y = relu(qs_d + qs_s) in cheap all-bf16 DVE ops. All partition bases
  are 32-aligned (HW engine constraint).
- PE warmup matmuls ramp the tensor-engine p-state while DMAs land.
  Slab chunks ride the SP HWDGE queue, weights the Pool SWDGE queue so
  their pre-transfer phases overlap.
- All compute bf16, PSUM accumulation f32.
"""
import numpy as np
import ml_dtypes

import concourse.bass as bass
import concourse.mybir as mybir
import concourse.bacc as bacc
import concourse.tile as tile
from concourse.bass_utils import run_bass_kernel_spmd

N_CORES = 8
NB = 256
BATCH = 2048
BF16 = mybir.dt.bfloat16
F32 = mybir.dt.float32
G = 4       # max i's per PSUM tile
NWARM = 5   # PE warmup matmuls (N=512 each)

# trimmed block configs: (cin, cout, k, s, noh, oh_full)
CFG = [(3, 4, 5, 3, 15, 20), (4, 6, 3, 2, 7, 9), (6, 16, 3, 2, 3, 4),
       (16, 32, 3, 2, 1, 1)]
JGS = [[(0, 8), (8, 7)], [(0, 5), (5, 2)], [(0, 2), (2, 1)], [(0, 1)]]
KS = [[79, 70], [61, 61], [45, 45], [49]]       # rhs partition count per jg
BIASROW = [[78, 69], [60, 60], [44, 44], [48]]  # bias row within the window
# SBUF weight tile shapes [K, noh, k, 64+S]
WSHAPES = {(0, 0): [79, 15, 5, 96], (0, 1): [70, 15, 5, 92],
           (1, 0): [61, 7, 3, 94], (1, 1): [61, 7, 3, 76],
           (2, 0): [45, 3, 3, 96], (2, 1): [45, 3, 3, 80],
           (3, 0): [49, 1, 3, 96]}

_CACHE = {}


def _y2row(w, c):
    """Row of value (w-index, channel) in the padded Y2 layout."""
    return w * 6 + c if w < 5 else 32 + (w - 5) * 6 + c


def _build():
    nc = bacc.Bacc("TRN2", target_bir_lowering=False, debug=False,
                   num_devices=N_CORES)
    xprep = nc.dram_tensor("xprep", [149, 47, NB], BF16, kind="ExternalInput").ap()
    wb = {}
    for key, shp in WSHAPES.items():
        wb[key] = nc.dram_tensor(f"wb{key[0]}_{key[1]}", shp, BF16,
                                 kind="ExternalInput").ap()
    wfc = nc.dram_tensor("wfc", [33, 4], BF16, kind="ExternalInput").ap()
    ones = nc.dram_tensor("ones", [1, 15 * NB], BF16, kind="ExternalInput").ap()
    out_d = nc.dram_tensor("out", [4, NB], F32, kind="ExternalOutput").ap()

    with tile.TileContext(nc) as tc:
        import contextlib
        ctx = contextlib.ExitStack()
        with ctx:
            pconst = ctx.enter_context(tc.tile_pool(name="const", bufs=1))
            pslab = ctx.enter_context(tc.tile_pool(name="slab", bufs=1))
            pg = ctx.enter_context(tc.tile_pool(name="g", bufs=4))
            pq = ctx.enter_context(tc.tile_pool(name="q", bufs=4))
            pp = ctx.enter_context(tc.tile_pool(name="p", bufs=4))
            pc = ctx.enter_context(tc.tile_pool(name="c", bufs=4))
            pps = ctx.enter_context(tc.tile_pool(name="ps", bufs=4, space="PSUM"))

            wfc_t = pconst.tile([33, 4], BF16, tag="wfc")
            out_t = pconst.tile([4, NB], F32, tag="outt")
            scratch = pconst.tile([64, 2, NB], BF16, tag="scratch")

            slab0 = pslab.tile([79, 47, NB], BF16, tag="slab0")
            slab1 = pslab.tile([70, 47, NB], BF16, tag="slab1")
            W = {key: pslab.tile(shp, BF16, tag=f"w{key[0]}_{key[1]}",
                                 name=f"w{key[0]}_{key[1]}")
                 for key, shp in WSHAPES.items()}
            Y1 = pslab.tile([61, 15, NB], BF16, tag="Y1")
            Y2 = pslab.tile([45, 7, NB], BF16, tag="Y2")
            Y3 = pslab.tile([49, 3, NB], BF16, tag="Y3")
            y4 = pslab.tile([33, 1, NB], BF16, tag="y4")
            SRC = [(slab0, slab1), (Y1, Y1), (Y2, Y2), (Y3,)]

            # ---- PE warmup: ramp the tensor engine p-state while DMAs land.
            nc.gpsimd.memset(scratch[:], 0.0)
            for _ in range(NWARM):
                warm = pps.tile([128, G, NB], F32, tag="acc")
                nc.tensor.matmul(warm[:, 0:2, :], scratch[:, 0, 0:128],
                                 scratch[:, 0:2, :], start=True, stop=True)

            # ---- DMA loads: slabs on the SP HWDGE queue, weights on the
            # Pool SWDGE queue; chunks aligned to i-group needs.
            HCH = [(0, 8), (8, 6), (14, 12), (26, 12), (38, 9)]
            ICH = [(0, 4), (4, 0), (4, 4), (8, 4), (12, 3)]
            # first weight chunks ride the SP queue ahead of the slabs so the
            # first matmul group's deps land earliest
            nc.sync.dma_start(W[(0, 0)][:, 0:4, :, :], wb[(0, 0)][:, 0:4, :, :])
            for t in range(5):
                h0, hn = HCH[t]
                i0, ni = ICH[t]
                nc.sync.dma_start(slab0[:, h0:h0 + hn, :],
                                  xprep[0:79, h0:h0 + hn, :])
                if t > 0 and ni:
                    nc.gpsimd.dma_start(W[(0, 0)][:, i0:i0 + ni, :, :],
                                        wb[(0, 0)][:, i0:i0 + ni, :, :])
                nc.sync.dma_start(slab1[:, h0:h0 + hn, :],
                                  xprep[79:149, h0:h0 + hn, :])
                if ni:
                    nc.gpsimd.dma_start(W[(0, 1)][:, i0:i0 + ni, :, :],
                                        wb[(0, 1)][:, i0:i0 + ni, :, :])

            for key in [(1, 0), (1, 1), (2, 0), (2, 1), (3, 0)]:
                nc.gpsimd.dma_start(W[key][:], wb[key][:])
            nc.sync.dma_start(wfc_t[:], wfc[:])
            # Y bias-constant rows via engine memsets in the idle startup
            # window (a DMA would land only after all slab chunks and stall
            # the interleaved tail groups).
            # engine APs need 32-aligned partition bases: memset the whole
            # [32:] range; the non-bias rows are overwritten by epilogues
            # before any consumer reads them.
            nc.vector.memset(Y1[32:61, :, :], 1.0)
            nc.vector.memset(Y2[32:45, :, :], 1.0)
            nc.vector.memset(Y3[32:49, :, :], 1.0)
            nc.vector.memset(y4[32:33, :, :], 1.0)

            ectr = [0]

            def epilogue(acc, S, g, ydst, split=False):
                """acc sections [d@0 | s@32 | g@64] of width S (32-aligned).
                gx = sigmoid(acc_g); q = gx*acc_d (PSUM+SBUF, equal base 0);
                p = q + s; y = relu(p). Block1 (split=True) alternates the
                s-section read between an ACT psum->sbuf copy (then cheap
                all-bf16 DVE add) and a DVE mixed-space psum add, and runs
                relu on Pool, spreading psum reads over three engines."""
                gx = pg.tile([32, G, NB], BF16, tag="g")
                qs = pq.tile([32, G, NB], BF16, tag="q")
                cs = pc.tile([32, G, NB], BF16, tag="c")
                p_t = pp.tile([32, G, NB], BF16, tag="p")
                nc.scalar.activation(gx[0:S, 0:g, :], acc[64:64 + S, 0:g, :],
                                     mybir.ActivationFunctionType.Sigmoid)
                use_act_copy = (not split) or (ectr[0] % 2 == 0)
                ectr[0] += 1
                if use_act_copy:
                    nc.scalar.activation(cs[0:S, 0:g, :], acc[32:32 + S, 0:g, :],
                                         mybir.ActivationFunctionType.Copy)
                nc.vector.tensor_tensor(qs[0:S, 0:g, :], gx[0:S, 0:g, :],
                                        acc[0:S, 0:g, :], mybir.AluOpType.mult)
                if use_act_copy:
                    nc.vector.tensor_tensor(p_t[0:S, 0:g, :], qs[0:S, 0:g, :],
                                            cs[0:S, 0:g, :],
                                            mybir.AluOpType.add)
                else:
                    nc.vector.tensor_tensor(p_t[0:S, 0:g, :], qs[0:S, 0:g, :],
                                            acc[32:32 + S, 0:g, :],
                                            mybir.AluOpType.add)
                if split:
                    nc.gpsimd.tensor_relu(ydst, p_t[0:S, 0:g, :])
                else:
                    nc.vector.tensor_relu(ydst, p_t[0:S, 0:g, :])

            # ---- blocks 1-4, software-pipelined schedule ----
            # Tail groups are interleaved into block1's back half as soon as
            # their Y-row dependencies allow, so they reuse PSUM ring slots
            # early and keep the PE p-state up (blk2 i-group (a,b) needs
            # block1 i <= 2*(a+b-1)+2; blk3 group i3 needs blk2 i2 <= 2*i3+2).
            YOUT = [Y1, Y2, Y3, y4]
            ROW0 = [[0, 32], [0, 32], [0, 32], [0]]
            SCHED = [(0, (0, 4)), (0, (4, 4)), (0, (8, 4)), (0, (12, 3)),
                     "fill",
                     (1, (0, 2)), (1, (2, 2)), (1, (4, 2)), (1, (6, 1)),
                     "fill",
                     (2, (0, 1)), (2, (1, 1)), (2, (2, 1)),
                     "fill",
                     (3, (0, 1))]

            def pe_filler(n):
                for _ in range(n):
                    warm = pps.tile([128, G, NB], F32, tag="acc")
                    nc.tensor.matmul(warm[:, 0:2, :], scratch[:, 0, 0:128],
                                     scratch[:, 0:2, :], start=True, stop=True)

            def emit_group(blk, i0, gn):
                cin, cout, k, s, noh, ohf = CFG[blk]
                for jg, (j0, nj) in enumerate(JGS[blk]):
                    S = nj * cout
                    K = KS[blk][jg]
                    src_t = SRC[blk][jg]
                    acc = pps.tile([128, G, NB], F32, tag="acc")
                    for ii in range(gn):
                        i = i0 + ii
                        for kh in range(k):
                            nc.tensor.matmul(
                                acc[0:64 + S, ii, :],
                                W[(blk, jg)][0:K, i, kh, :],
                                src_t[0:K, s * i + kh, :],
                                start=(kh == 0), stop=(kh == k - 1))
                    yt = YOUT[blk]
                    r0 = ROW0[blk][jg]
                    if blk == 3:
                        ydst = yt[0:32, 0:1, :]
                    else:
                        ydst = yt[r0:r0 + S, i0:i0 + gn, :]
                    epilogue(acc, S, gn, ydst, split=(blk == 0))

            for item in SCHED:
                if item == "fill":
                    pe_filler(2)
                else:
                    emit_group(item[0], *item[1])

            # ---- FC ----
            accfc = pps.tile([128, G, NB], F32, tag="acc")
            nc.tensor.matmul(accfc[0:4, 0, :], wfc_t[:], y4[0:33, 0, :],
                             start=True, stop=True)
            nc.scalar.activation(out_t[:], accfc[0:4, 0, :],
                                 mybir.ActivationFunctionType.Copy)
            nc.sync.dma_start(out_d[:], out_t[:])

    nc.compile()
    return nc


def _prep_weights(inputs):
    """Fused block-diag weight tensors: 32-aligned sections [d|s|g],
    bias rows folded in (the u rows come from the +30 matmul)."""
    arrs = {}
    for blk in range(4):
        cin, cout, k, s, noh, ohf = CFG[blk]
        wu = np.asarray(inputs[f"w_uc{blk + 1}"], np.float32).reshape(
            ohf * ohf, cin * k * k, cout)
        bu = np.asarray(inputs[f"b_uc{blk + 1}"], np.float32)[0]
        wp = np.asarray(inputs[f"w_pc{blk + 1}"], np.float32)
        bp = np.asarray(inputs[f"b_pc{blk + 1}"], np.float32)
        wg = np.asarray(inputs[f"w_wl{blk + 1}"], np.float32)[0]
        bg = float(np.asarray(inputs[f"b_wl{blk + 1}"], np.float32)[0])

        for jg, (j0, nj) in enumerate(JGS[blk]):
            S = nj * cout
            K = KS[blk][jg]
            brow = BIASROW[blk][jg]
            if blk == 0:
                w0 = 0 if jg == 0 else 24
                rowf = lambda w, c: (w - w0) * 3 + c
            elif blk == 1:
                rowf = lambda w, c: w * 4 + c
            elif blk == 2:
                rowf = lambda w, c: _y2row(w, c)
            else:
                rowf = lambda w, c: w * 16 + c
            Wt = np.zeros((K, noh, k, 64 + S), np.float32)
            ivec = np.arange(noh)
            for jt in range(nj):
                j = j0 + jt
                c0, c1, c2 = jt * cout, 32 + jt * cout, 64 + jt * cout
                for kw in range(k):
                    w = s * j + kw
                    for c in range(cin):
                        row = rowf(w, c)
                        for kh in range(k):
                            un = wu[ivec * ohf + j, c * k * k + kh * k + kw, :]
                            Wt[row, :, kh, c0:c0 + cout] = un - wp[:, c, kh, kw]
                            Wt[row, :, kh, c1:c1 + cout] = wp[:, c, kh, kw]
                            Wt[row, :, kh, c2:c2 + cout] = wg[c, kh, kw]
                Wt[brow, :, 0, c0:c0 + cout] = bu[:, ivec, j].T - bp
                Wt[brow, :, 0, c1:c1 + cout] = bp
                Wt[brow, :, 0, c2:c2 + cout] = bg
            arrs[f"wb{blk}_{jg}"] = Wt.astype(ml_dtypes.bfloat16)

    wfc = np.zeros((33, 4), np.float32)
    wfc[0:32] = np.asarray(inputs["fc_w"], np.float32)
    wfc[32] = np.asarray(inputs["fc_b"], np.float32)
    arrs["wfc"] = wfc.astype(ml_dtypes.bfloat16)
    arrs["ones"] = np.ones((1, 15 * NB), ml_dtypes.bfloat16)
    return arrs


def make_in_maps(inputs):
    warrs = _prep_weights(inputs)
    x = np.asarray(inputs["x"], np.float32)
    # [w, c, h, b] view of the live x region, rows (w*3+c)
    xt = np.ascontiguousarray(x[:, :, 0:47, 0:47].transpose(3, 1, 2, 0))
    xt = xt.reshape(141, 47, BATCH).astype(ml_dtypes.bfloat16)
    in_maps = []
    for ci in range(N_CORES):
        xc = xt[:, :, ci * NB:(ci + 1) * NB]
        xprep = np.empty((149, 47, NB), ml_dtypes.bfloat16)
        xprep[0:78] = xc[0:78]          # slab0: w 0..25
        xprep[78] = 1.0
        xprep[79:148] = xc[72:141]      # slab1: w 24..46
        xprep[148] = 1.0
        m = {"xprep": xprep}
        m.update(warrs)
        in_maps.append(m)
    return in_maps


def kernel(**inputs):
    if "nc" not in _CACHE:
        _CACHE["nc"] = _build()
    nc = _CACHE["nc"]
    in_maps = make_in_maps(inputs)
    res = run_bass_kernel_spmd(nc, in_maps, core_ids=list(range(N_CORES)))
    out = np.concatenate([res.results[c]["out"].T for c in range(N_CORES)], axis=0)
    return out.astype(np.float32)


# revision 71
# speedup vs baseline: 1.1137x; 1.0270x over previous
"""BlockNet Trainium2 kernel: data-parallel over 8 NeuronCores.

Design (per core, batch NB=256):
- VALID-conv pyramid trimming: the final [B,4] output only depends on
  x[:, :, 0:47, 0:47], block1 out (i,j)<=14, block2 out <=6, block3 out <=2.
  Everything else is dead and never computed or loaded.
- Host pre-transposes x to xprep[(w,c), h, b] bf16 with constant-1 bias
  rows; all conv biases fold into the matmuls via an extra weight row.
- Each block's untied+shared+gate convs run as ONE fused matmul per
  (jg, i, kh): lhsT [K, 64+S] block-diagonal over j with 32-aligned
  sections [d@0 | s@32 | g@64] (S = nj*cout <= 32), rhs = slab [K, 256]
  at h = s*i+kh, accumulated over kh into a PSUM tile batching G i's.
  A 1-row matmul writes +30 into [96:96+S] so sigmoid gives exact 1.0
  rows ("u" section).
- Epilogue per i-group: gx = sigmoid(acc[64:128]) = [g'; 1] (ACT);
  qs = gx * acc[0:64] = [g'*d ; s] (one PSUM-reading DVE op);
  y = relu(qs_d + qs_s) in cheap all-bf16 DVE ops. All partition bases
  are 32-aligned (HW engine constraint).
- PE warmup matmuls ramp the tensor-engine p-state while DMAs land.
  Slab chunks ride the SP HWDGE queue, weights the Pool SWDGE queue so
  their pre-transfer phases overlap.
- All compute bf16, PSUM accumulation f32.
"""
import numpy as np
import ml_dtypes

import concourse.bass as bass
import concourse.mybir as mybir
import concourse.bacc as bacc
import concourse.tile as tile
from concourse.bass_utils import run_bass_kernel_spmd

N_CORES = 8
NB = 256
BATCH = 2048
BF16 = mybir.dt.bfloat16
F32 = mybir.dt.float32
G = 4       # max i's per PSUM tile
NWARM = 5   # PE warmup matmuls (N=512 each)

# trimmed block configs: (cin, cout, k, s, noh, oh_full)
CFG = [(3, 4, 5, 3, 15, 20), (4, 6, 3, 2, 7, 9), (6, 16, 3, 2, 3, 4),
       (16, 32, 3, 2, 1, 1)]
JGS = [[(0, 8), (8, 7)], [(0, 5), (5, 2)], [(0, 2), (2, 1)], [(0, 1)]]
KS = [[79, 70], [61, 61], [45, 45], [49]]       # rhs partition count per jg
BIASROW = [[78, 69], [60, 60], [44, 44], [48]]  # bias row within the window
# SBUF weight tile shapes [K, noh, k, 64+S]
WSHAPES = {(0, 0): [79, 15, 5, 96], (0, 1): [70, 15, 5, 92],
           (1, 0): [61, 7, 3, 94], (1, 1): [61, 7, 3, 76],
           (2, 0): [45, 3, 3, 96], (2, 1): [45, 3, 3, 80],
           (3, 0): [49, 1, 3, 96]}

_CACHE = {}


def _y2row(w, c):
    """Row of value (w-index, channel) in the padded Y2 layout."""
    return w * 6 + c if w < 5 else 32 + (w - 5) * 6 + c


def _build():
    nc = bacc.Bacc("TRN2", target_bir_lowering=False, debug=False,
                   num_devices=N_CORES)
    xprep = nc.dram_tensor("xprep", [149, 47, NB], BF16, kind="ExternalInput").ap()
    wb = {}
    for key, shp in WSHAPES.items():
        wb[key] = nc.dram_tensor(f"wb{key[0]}_{key[1]}", shp, BF16,
                                 kind="ExternalInput").ap()
    wfc = nc.dram_tensor("wfc", [33, 4], BF16, kind="ExternalInput").ap()
    ones = nc.dram_tensor("ones", [1, 15 * NB], BF16, kind="ExternalInput").ap()
    out_d = nc.dram_tensor("out", [4, NB], F32, kind="ExternalOutput").ap()

    with tile.TileContext(nc) as tc:
        import contextlib
        ctx = contextlib.ExitStack()
        with ctx:
            pconst = ctx.enter_context(tc.tile_pool(name="const", bufs=1))
            pslab = ctx.enter_context(tc.tile_pool(name="slab", bufs=1))
            pg = ctx.enter_context(tc.tile_pool(name="g", bufs=4))
            pq = ctx.enter_context(tc.tile_pool(name="q", bufs=4))
            pp = ctx.enter_context(tc.tile_pool(name="p", bufs=4))
            pc = ctx.enter_context(tc.tile_pool(name="c", bufs=4))
            pps = ctx.enter_context(tc.tile_pool(name="ps", bufs=4, space="PSUM"))

            wfc_t = pconst.tile([33, 4], BF16, tag="wfc")
            out_t = pconst.tile([4, NB], F32, tag="outt")
            scratch = pconst.tile([64, 2, NB], BF16, tag="scratch")

            slab0 = pslab.tile([79, 47, NB], BF16, tag="slab0")
            slab1 = pslab.tile([70, 47, NB], BF16, tag="slab1")
            W = {key: pslab.tile(shp, BF16, tag=f"w{key[0]}_{key[1]}",
                                 name=f"w{key[0]}_{key[1]}")
                 for key, shp in WSHAPES.items()}
            Y1 = pslab.tile([61, 15, NB], BF16, tag="Y1")
            Y2 = pslab.tile([45, 7, NB], BF16, tag="Y2")
            Y3 = pslab.tile([49, 3, NB], BF16, tag="Y3")
            y4 = pslab.tile([33, 1, NB], BF16, tag="y4")
            SRC = [(slab0, slab1), (Y1, Y1), (Y2, Y2), (Y3,)]

            # ---- PE warmup: ramp the tensor engine p-state while DMAs land.
            nc.gpsimd.memset(scratch[:], 0.0)
            for _ in range(NWARM):
                warm = pps.tile([128, G, NB], F32, tag="acc")
                nc.tensor.matmul(warm[:, 0:2, :], scratch[:, 0, 0:128],
                                 scratch[:, 0:2, :], start=True, stop=True)

            # ---- DMA loads: slabs on the SP HWDGE queue, weights on the
            # Pool SWDGE queue; chunks aligned to i-group needs.
            HCH = [(0, 8), (8, 6), (14, 12), (26, 12), (38, 9)]
            ICH = [(0, 4), (4, 0), (4, 4), (8, 4), (12, 3)]
            # first weight chunks ride the SP queue ahead of the slabs so the
            # first matmul group's deps land earliest
            nc.sync.dma_start(W[(0, 0)][:, 0:4, :, :], wb[(0, 0)][:, 0:4, :, :])
            for t in range(5):
                h0, hn = HCH[t]
                i0, ni = ICH[t]
                nc.sync.dma_start(slab0[:, h0:h0 + hn, :],
                                  xprep[0:79, h0:h0 + hn, :])
                if t > 0 and ni:
                    nc.gpsimd.dma_start(W[(0, 0)][:, i0:i0 + ni, :, :],
                                        wb[(0, 0)][:, i0:i0 + ni, :, :])
                nc.sync.dma_start(slab1[:, h0:h0 + hn, :],
                                  xprep[79:149, h0:h0 + hn, :])
                if ni:
                    nc.gpsimd.dma_start(W[(0, 1)][:, i0:i0 + ni, :, :],
                                        wb[(0, 1)][:, i0:i0 + ni, :, :])

            for key in [(1, 0), (1, 1), (2, 0), (2, 1), (3, 0)]:
                nc.gpsimd.dma_start(W[key][:], wb[key][:])
            nc.sync.dma_start(wfc_t[:], wfc[:])
            # Y bias-constant rows via engine memsets in the idle startup
            # window (a DMA would land only after all slab chunks and stall
            # the interleaved tail groups).
            # engine APs need 32-aligned partition bases: memset the whole
            # [32:] range; the non-bias rows are overwritten by epilogues
            # before any consumer reads them.
            nc.vector.memset(Y1[32:61, :, :], 1.0)
            nc.vector.memset(Y2[32:45, :, :], 1.0)
            nc.vector.memset(Y3[32:49, :, :], 1.0)
            nc.vector.memset(y4[32:33, :, :], 1.0)

            ectr = [0]

            def epilogue(acc, S, g, ydst, split=False):
                """acc sections [d@0 | s@32 | g@64] of width S (32-aligned).
                gx = sigmoid(acc_g); q = gx*acc_d (PSUM+SBUF, equal base 0);
                p = q + s; y = relu(p). Block1 (split=True) alternates the
                s-section read between an ACT psum->sbuf copy (then cheap
                all-bf16 DVE add) and a DVE mixed-space psum add, and runs
                relu on Pool, spreading psum reads over three engines."""
                gx = pg.tile([32, G, NB], BF16, tag="g")
                qs = pq.tile([32, G, NB], BF16, tag="q")
                p_t = pp.tile([32, G, NB], BF16, tag="p")
                nc.scalar.activation(gx[0:S, 0:g, :], acc[64:64 + S, 0:g, :],
                                     mybir.ActivationFunctionType.Sigmoid)
                use_act_copy = (not split) or (ectr[0] % 3 != 2)
                ectr[0] += 1
                if use_act_copy:
                    cs = pc.tile([32, G, NB], BF16, tag="c")
                    nc.scalar.activation(cs[0:S, 0:g, :], acc[32:32 + S, 0:g, :],
                                         mybir.ActivationFunctionType.Copy)
                nc.vector.tensor_tensor(qs[0:S, 0:g, :], gx[0:S, 0:g, :],
                                        acc[0:S, 0:g, :], mybir.AluOpType.mult)
                if use_act_copy:
                    nc.vector.tensor_tensor(p_t[0:S, 0:g, :], qs[0:S, 0:g, :],
                                            cs[0:S, 0:g, :],
                                            mybir.AluOpType.add)
                else:
                    nc.vector.tensor_tensor(p_t[0:S, 0:g, :], qs[0:S, 0:g, :],
                                            acc[32:32 + S, 0:g, :],
                                            mybir.AluOpType.add)
                if split:
                    nc.gpsimd.tensor_relu(ydst, p_t[0:S, 0:g, :])
                else:
                    nc.vector.tensor_relu(ydst, p_t[0:S, 0:g, :])

            # ---- blocks 1-4, software-pipelined schedule ----
            # Tail groups are interleaved into block1's back half as soon as
            # their Y-row dependencies allow, so they reuse PSUM ring slots
            # early and keep the PE p-state up (blk2 i-group (a,b) needs
            # block1 i <= 2*(a+b-1)+2; blk3 group i3 needs blk2 i2 <= 2*i3+2).
            YOUT = [Y1, Y2, Y3, y4]
            ROW0 = [[0, 32], [0, 32], [0, 32], [0]]
            SCHED = [(0, (0, 4)), (0, (4, 4)), (0, (8, 4)), (0, (12, 3)),
                     "fill",
                     (1, (0, 2)), (1, (2, 2)), (1, (4, 2)), (1, (6, 1)),
                     "fill",
                     (2, (0, 1)), (2, (1, 1)), (2, (2, 1)),
                     "fill",
                     (3, (0, 1))]

            def pe_filler(n):
                for _ in range(n):
                    warm = pps.tile([128, G, NB], F32, tag="acc")
                    nc.tensor.matmul(warm[:, 0:2, :], scratch[:, 0, 0:128],
                                     scratch[:, 0:2, :], start=True, stop=True)

            def emit_group(blk, i0, gn):
                cin, cout, k, s, noh, ohf = CFG[blk]
                for jg, (j0, nj) in enumerate(JGS[blk]):
                    S = nj * cout
                    K = KS[blk][jg]
                    src_t = SRC[blk][jg]
                    acc = pps.tile([128, G, NB], F32, tag="acc")
                    for ii in range(gn):
                        i = i0 + ii
                        for kh in range(k):
                            nc.tensor.matmul(
                                acc[0:64 + S, ii, :],
                                W[(blk, jg)][0:K, i, kh, :],
                                src_t[0:K, s * i + kh, :],
                                start=(kh == 0), stop=(kh == k - 1))
                    yt = YOUT[blk]
                    r0 = ROW0[blk][jg]
                    if blk == 3:
                        ydst = yt[0:32, 0:1, :]
                    else:
                        ydst = yt[r0:r0 + S, i0:i0 + gn, :]
                    epilogue(acc, S, gn, ydst, split=(blk == 0))

            for item in SCHED:
                if item == "fill":
                    pe_filler(2)
                else:
                    emit_group(item[0], *item[1])

            # ---- FC ----
            accfc = pps.tile([128, G, NB], F32, tag="acc")
            nc.tensor.matmul(accfc[0:4, 0, :], wfc_t[:], y4[0:33, 0, :],
                             start=True, stop=True)
            nc.scalar.activation(out_t[:], accfc[0:4, 0, :],
                                 mybir.ActivationFunctionType.Copy)
            nc.sync.dma_start(out_d[:], out_t[:])

    nc.compile()
    return nc


def _prep_weights(inputs):
    """Fused block-diag weight tensors: 32-aligned sections [d|s|g],
    bias rows folded in (the u rows come from the +30 matmul)."""
    arrs = {}
    for blk in range(4):
        cin, cout, k, s, noh, ohf = CFG[blk]
        wu = np.asarray(inputs[f"w_uc{blk + 1}"], np.float32).reshape(
            ohf * ohf, cin * k * k, cout)
        bu = np.asarray(inputs[f"b_uc{blk + 1}"], np.float32)[0]
        wp = np.asarray(inputs[f"w_pc{blk + 1}"], np.float32)
        bp = np.asarray(inputs[f"b_pc{blk + 1}"], np.float32)
        wg = np.asarray(inputs[f"w_wl{blk + 1}"], np.float32)[0]
        bg = float(np.asarray(inputs[f"b_wl{blk + 1}"], np.float32)[0])

        for jg, (j0, nj) in enumerate(JGS[blk]):
            S = nj * cout
            K = KS[blk][jg]
            brow = BIASROW[blk][jg]
            if blk == 0:
                w0 = 0 if jg == 0 else 24
                rowf = lambda w, c: (w - w0) * 3 + c
            elif blk == 1:
                rowf = lambda w, c: w * 4 + c
            elif blk == 2:
                rowf = lambda w, c: _y2row(w, c)
            else:
                rowf = lambda w, c: w * 16 + c
            Wt = np.zeros((K, noh, k, 64 + S), np.float32)
            ivec = np.arange(noh)
            for jt in range(nj):
                j = j0 + jt
                c0, c1, c2 = jt * cout, 32 + jt * cout, 64 + jt * cout
                for kw in range(k):
                    w = s * j + kw
                    for c in range(cin):
                        row = rowf(w, c)
                        for kh in range(k):
                            un = wu[ivec * ohf + j, c * k * k + kh * k + kw, :]
                            Wt[row, :, kh, c0:c0 + cout] = un - wp[:, c, kh, kw]
                            Wt[row, :, kh, c1:c1 + cout] = wp[:, c, kh, kw]
                            Wt[row, :, kh, c2:c2 + cout] = wg[c, kh, kw]
                Wt[brow, :, 0, c0:c0 + cout] = bu[:, ivec, j].T - bp
                Wt[brow, :, 0, c1:c1 + cout] = bp
                Wt[brow, :, 0, c2:c2 + cout] = bg
            arrs[f"wb{blk}_{jg}"] = Wt.astype(ml_dtypes.bfloat16)

    wfc = np.zeros((33, 4), np.float32)
    wfc[0:32] = np.asarray(inputs["fc_w"], np.float32)
    wfc[32] = np.asarray(inputs["fc_b"], np.float32)
    arrs["wfc"] = wfc.astype(ml_dtypes.bfloat16)
    arrs["ones"] = np.ones((1, 15 * NB), ml_dtypes.bfloat16)
    return arrs


def make_in_maps(inputs):
    warrs = _prep_weights(inputs)
    x = np.asarray(inputs["x"], np.float32)
    # [w, c, h, b] view of the live x region, rows (w*3+c)
    xt = np.ascontiguousarray(x[:, :, 0:47, 0:47].transpose(3, 1, 2, 0))
    xt = xt.reshape(141, 47, BATCH).astype(ml_dtypes.bfloat16)
    in_maps = []
    for ci in range(N_CORES):
        xc = xt[:, :, ci * NB:(ci + 1) * NB]
        xprep = np.empty((149, 47, NB), ml_dtypes.bfloat16)
        xprep[0:78] = xc[0:78]          # slab0: w 0..25
        xprep[78] = 1.0
        xprep[79:148] = xc[72:141]      # slab1: w 24..46
        xprep[148] = 1.0
        m = {"xprep": xprep}
        m.update(warrs)
        in_maps.append(m)
    return in_maps


def kernel(**inputs):
    if "nc" not in _CACHE:
        _CACHE["nc"] = _build()
    nc = _CACHE["nc"]
    in_maps = make_in_maps(inputs)
    res = run_bass_kernel_spmd(nc, in_maps, core_ids=list(range(N_CORES)))
    out = np.concatenate([res.results[c]["out"].T for c in range(N_CORES)], axis=0)
    return out.astype(np.float32)


# revision 78
# speedup vs baseline: 1.1224x; 1.0078x over previous
"""BlockNet Trainium2 kernel: data-parallel over 8 NeuronCores.

Design (per core, batch NB=256):
- VALID-conv pyramid trimming: the final [B,4] output only depends on
  x[:, :, 0:47, 0:47], block1 out (i,j)<=14, block2 out <=6, block3 out <=2.
  Everything else is dead and never computed or loaded.
- Host pre-transposes x to xprep[(w,c), h, b] bf16 with constant-1 bias
  rows; all conv biases fold into the matmuls via an extra weight row.
- Each block's untied+shared+gate convs run as ONE fused matmul per
  (jg, i, kh): lhsT [K, 64+S] block-diagonal over j with 32-aligned
  sections [d@0 | s@32 | g@64] (S = nj*cout <= 32), rhs = slab [K, 256]
  at h = s*i+kh, accumulated over kh into a PSUM tile batching G i's.
  A 1-row matmul writes +30 into [96:96+S] so sigmoid gives exact 1.0
  rows ("u" section).
- Epilogue per i-group: gx = sigmoid(acc[64:128]) = [g'; 1] (ACT);
  qs = gx * acc[0:64] = [g'*d ; s] (one PSUM-reading DVE op);
  y = relu(qs_d + qs_s) in cheap all-bf16 DVE ops. All partition bases
  are 32-aligned (HW engine constraint).
- PE warmup matmuls ramp the tensor-engine p-state while DMAs land.
  Slab chunks ride the SP HWDGE queue, weights the Pool SWDGE queue so
  their pre-transfer phases overlap.
- All compute bf16, PSUM accumulation f32.
"""
import numpy as np
import ml_dtypes

import concourse.bass as bass
import concourse.mybir as mybir
import concourse.bacc as bacc
import concourse.tile as tile
from concourse.bass_utils import run_bass_kernel_spmd

N_CORES = 8
NB = 256
BATCH = 2048
BF16 = mybir.dt.bfloat16
F32 = mybir.dt.float32
G = 4       # max i's per PSUM tile
NWARM = 5   # PE warmup matmuls (N=512 each)

# trimmed block configs: (cin, cout, k, s, noh, oh_full)
CFG = [(3, 4, 5, 3, 15, 20), (4, 6, 3, 2, 7, 9), (6, 16, 3, 2, 3, 4),
       (16, 32, 3, 2, 1, 1)]
JGS = [[(0, 8), (8, 7)], [(0, 5), (5, 2)], [(0, 2), (2, 1)], [(0, 1)]]
KS = [[79, 70], [61, 61], [45, 45], [49]]       # rhs partition count per jg
BIASROW = [[78, 69], [60, 60], [44, 44], [48]]  # bias row within the window
# SBUF weight tile shapes [K, noh, k, 64+S]
WSHAPES = {(0, 0): [79, 15, 5, 96], (0, 1): [70, 15, 5, 92],
           (1, 0): [61, 7, 3, 94], (1, 1): [61, 7, 3, 76],
           (2, 0): [45, 3, 3, 96], (2, 1): [45, 3, 3, 80],
           (3, 0): [49, 1, 3, 96]}

_CACHE = {}


def _y2row(w, c):
    """Row of value (w-index, channel) in the padded Y2 layout."""
    return w * 6 + c if w < 5 else 32 + (w - 5) * 6 + c


def _build():
    nc = bacc.Bacc("TRN2", target_bir_lowering=False, debug=False,
                   num_devices=N_CORES)
    xprep = nc.dram_tensor("xprep", [149, 47, NB], BF16, kind="ExternalInput").ap()
    wb = {}
    for key, shp in WSHAPES.items():
        wb[key] = nc.dram_tensor(f"wb{key[0]}_{key[1]}", shp, BF16,
                                 kind="ExternalInput").ap()
    wfc = nc.dram_tensor("wfc", [33, 4], BF16, kind="ExternalInput").ap()
    ones = nc.dram_tensor("ones", [1, 15 * NB], BF16, kind="ExternalInput").ap()
    out_d = nc.dram_tensor("out", [4, NB], F32, kind="ExternalOutput").ap()

    with tile.TileContext(nc) as tc:
        import contextlib
        ctx = contextlib.ExitStack()
        with ctx:
            pconst = ctx.enter_context(tc.tile_pool(name="const", bufs=1))
            pslab = ctx.enter_context(tc.tile_pool(name="slab", bufs=1))
            pg = ctx.enter_context(tc.tile_pool(name="g", bufs=4))
            pq = ctx.enter_context(tc.tile_pool(name="q", bufs=4))
            pp = ctx.enter_context(tc.tile_pool(name="p", bufs=4))
            pc = ctx.enter_context(tc.tile_pool(name="c", bufs=4))
            pps = ctx.enter_context(tc.tile_pool(name="ps", bufs=4, space="PSUM"))

            wfc_t = pconst.tile([33, 4], BF16, tag="wfc")
            out_t = pconst.tile([4, NB], F32, tag="outt")
            scratch = pconst.tile([64, 2, NB], BF16, tag="scratch")

            slab0 = pslab.tile([79, 47, NB], BF16, tag="slab0")
            slab1 = pslab.tile([70, 47, NB], BF16, tag="slab1")
            W = {key: pslab.tile(shp, BF16, tag=f"w{key[0]}_{key[1]}",
                                 name=f"w{key[0]}_{key[1]}")
                 for key, shp in WSHAPES.items()}
            Y1 = pslab.tile([61, 15, NB], BF16, tag="Y1")
            Y2 = pslab.tile([45, 7, NB], BF16, tag="Y2")
            Y3 = pslab.tile([49, 3, NB], BF16, tag="Y3")
            y4 = pslab.tile([33, 1, NB], BF16, tag="y4")
            SRC = [(slab0, slab1), (Y1, Y1), (Y2, Y2), (Y3,)]

            # ---- PE warmup: ramp the tensor engine p-state while DMAs land.
            nc.gpsimd.memset(scratch[:], 0.0)
            for _ in range(NWARM):
                warm = pps.tile([128, G, NB], F32, tag="acc")
                nc.tensor.matmul(warm[:, 0:2, :], scratch[:, 0, 0:128],
                                 scratch[:, 0:2, :], start=True, stop=True)

            # ---- DMA loads: slabs on the SP HWDGE queue, weights on the
            # Pool SWDGE queue; chunks aligned to i-group needs.
            HCH = [(0, 8), (8, 6), (14, 12), (26, 12), (38, 9)]
            ICH = [(0, 4), (4, 0), (4, 4), (8, 4), (12, 3)]
            # first weight chunks ride the SP queue ahead of the slabs so the
            # first matmul group's deps land earliest
            nc.sync.dma_start(W[(0, 0)][:, 0:4, :, :], wb[(0, 0)][:, 0:4, :, :])
            for t in range(5):
                h0, hn = HCH[t]
                i0, ni = ICH[t]
                nc.sync.dma_start(slab0[:, h0:h0 + hn, :],
                                  xprep[0:79, h0:h0 + hn, :])
                if t > 0 and ni:
                    nc.gpsimd.dma_start(W[(0, 0)][:, i0:i0 + ni, :, :],
                                        wb[(0, 0)][:, i0:i0 + ni, :, :])
                nc.sync.dma_start(slab1[:, h0:h0 + hn, :],
                                  xprep[79:149, h0:h0 + hn, :])
                if ni:
                    nc.gpsimd.dma_start(W[(0, 1)][:, i0:i0 + ni, :, :],
                                        wb[(0, 1)][:, i0:i0 + ni, :, :])

            for key in [(1, 0), (1, 1), (2, 0), (2, 1), (3, 0)]:
                nc.gpsimd.dma_start(W[key][:], wb[key][:])
            nc.sync.dma_start(wfc_t[:], wfc[:])
            # Y bias-constant rows via engine memsets in the idle startup
            # window (a DMA would land only after all slab chunks and stall
            # the interleaved tail groups).
            # engine APs need 32-aligned partition bases: memset the whole
            # [32:] range; the non-bias rows are overwritten by epilogues
            # before any consumer reads them.
            nc.vector.memset(Y1[32:61, :, :], 1.0)
            nc.vector.memset(Y2[32:45, :, :], 1.0)
            nc.vector.memset(Y3[32:49, :, :], 1.0)
            nc.vector.memset(y4[32:33, :, :], 1.0)

            ectr = [0]

            def epilogue(acc, S, g, ydst, split=False, nocopy=False):
                """acc sections [d@0 | s@32 | g@64] of width S (32-aligned).
                gx = sigmoid(acc_g); q = gx*acc_d (PSUM+SBUF, equal base 0);
                p = q + s; y = relu(p). Block1 (split=True) alternates the
                s-section read between an ACT psum->sbuf copy (then cheap
                all-bf16 DVE add) and a DVE mixed-space psum add, and runs
                relu on Pool, spreading psum reads over three engines."""
                gx = pg.tile([32, G, NB], BF16, tag="g")
                qs = pq.tile([32, G, NB], BF16, tag="q")
                p_t = pp.tile([32, G, NB], BF16, tag="p")
                nc.scalar.activation(gx[0:S, 0:g, :], acc[64:64 + S, 0:g, :],
                                     mybir.ActivationFunctionType.Sigmoid)
                use_act_copy = (not nocopy) and ((not split) or (ectr[0] % 3 != 2))
                ectr[0] += 1
                if use_act_copy:
                    cs = pc.tile([32, G, NB], BF16, tag="c")
                    nc.scalar.activation(cs[0:S, 0:g, :], acc[32:32 + S, 0:g, :],
                                         mybir.ActivationFunctionType.Copy)
                nc.vector.tensor_tensor(qs[0:S, 0:g, :], gx[0:S, 0:g, :],
                                        acc[0:S, 0:g, :], mybir.AluOpType.mult)
                if use_act_copy:
                    nc.vector.tensor_tensor(p_t[0:S, 0:g, :], qs[0:S, 0:g, :],
                                            cs[0:S, 0:g, :],
                                            mybir.AluOpType.add)
                else:
                    nc.vector.tensor_tensor(p_t[0:S, 0:g, :], qs[0:S, 0:g, :],
                                            acc[32:32 + S, 0:g, :],
                                            mybir.AluOpType.add)
                if split:
                    nc.gpsimd.tensor_relu(ydst, p_t[0:S, 0:g, :])
                else:
                    nc.vector.tensor_relu(ydst, p_t[0:S, 0:g, :])

            # ---- blocks 1-4, software-pipelined schedule ----
            # Tail groups are interleaved into block1's back half as soon as
            # their Y-row dependencies allow, so they reuse PSUM ring slots
            # early and keep the PE p-state up (blk2 i-group (a,b) needs
            # block1 i <= 2*(a+b-1)+2; blk3 group i3 needs blk2 i2 <= 2*i3+2).
            YOUT = [Y1, Y2, Y3, y4]
            ROW0 = [[0, 32], [0, 32], [0, 32], [0]]
            SCHED = [(0, (0, 4)), (0, (4, 4)), (0, (8, 4)), (0, (12, 3)),
                     "fill",
                     (1, (0, 2)), (1, (2, 2)), (1, (4, 2)), (1, (6, 1)),
                     "fill",
                     (2, (0, 1)), (2, (1, 1)), (2, (2, 1)),
                     "fill",
                     (3, (0, 1))]

            def pe_filler(n):
                for _ in range(n):
                    warm = pps.tile([128, G, NB], F32, tag="acc")
                    nc.tensor.matmul(warm[:, 0:2, :], scratch[:, 0, 0:128],
                                     scratch[:, 0:2, :], start=True, stop=True)

            def emit_group(blk, i0, gn):
                cin, cout, k, s, noh, ohf = CFG[blk]
                for jg, (j0, nj) in enumerate(JGS[blk]):
                    S = nj * cout
                    K = KS[blk][jg]
                    src_t = SRC[blk][jg]
                    acc = pps.tile([128, G, NB], F32, tag="acc")
                    for ii in range(gn):
                        i = i0 + ii
                        for kh in range(k):
                            nc.tensor.matmul(
                                acc[0:64 + S, ii, :],
                                W[(blk, jg)][0:K, i, kh, :],
                                src_t[0:K, s * i + kh, :],
                                start=(kh == 0), stop=(kh == k - 1))
                    yt = YOUT[blk]
                    r0 = ROW0[blk][jg]
                    if blk == 3:
                        ydst = yt[0:32, 0:1, :]
                    else:
                        ydst = yt[r0:r0 + S, i0:i0 + gn, :]
                    epilogue(acc, S, gn, ydst, split=(blk == 0), nocopy=(blk == 3))

            for item in SCHED:
                if item == "fill":
                    pe_filler(2)
                else:
                    emit_group(item[0], *item[1])

            # ---- FC ----
            accfc = pps.tile([128, G, NB], F32, tag="acc")
            nc.tensor.matmul(accfc[0:4, 0, :], wfc_t[:], y4[0:33, 0, :],
                             start=True, stop=True)
            nc.scalar.activation(out_t[:], accfc[0:4, 0, :],
                                 mybir.ActivationFunctionType.Copy)
            nc.sync.dma_start(out_d[:], out_t[:])

    nc.compile()
    return nc


def _prep_weights(inputs):
    """Fused block-diag weight tensors: 32-aligned sections [d|s|g],
    bias rows folded in (the u rows come from the +30 matmul)."""
    arrs = {}
    for blk in range(4):
        cin, cout, k, s, noh, ohf = CFG[blk]
        wu = np.asarray(inputs[f"w_uc{blk + 1}"], np.float32).reshape(
            ohf * ohf, cin * k * k, cout)
        bu = np.asarray(inputs[f"b_uc{blk + 1}"], np.float32)[0]
        wp = np.asarray(inputs[f"w_pc{blk + 1}"], np.float32)
        bp = np.asarray(inputs[f"b_pc{blk + 1}"], np.float32)
        wg = np.asarray(inputs[f"w_wl{blk + 1}"], np.float32)[0]
        bg = float(np.asarray(inputs[f"b_wl{blk + 1}"], np.float32)[0])

        for jg, (j0, nj) in enumerate(JGS[blk]):
            S = nj * cout
            K = KS[blk][jg]
            brow = BIASROW[blk][jg]
            if blk == 0:
                w0 = 0 if jg == 0 else 24
                rowf = lambda w, c: (w - w0) * 3 + c
            elif blk == 1:
                rowf = lambda w, c: w * 4 + c
            elif blk == 2:
                rowf = lambda w, c: _y2row(w, c)
            else:
                rowf = lambda w, c: w * 16 + c
            Wt = np.zeros((K, noh, k, 64 + S), np.float32)
            ivec = np.arange(noh)
            for jt in range(nj):
                j = j0 + jt
                c0, c1, c2 = jt * cout, 32 + jt * cout, 64 + jt * cout
                for kw in range(k):
                    w = s * j + kw
                    for c in range(cin):
                        row = rowf(w, c)
                        for kh in range(k):
                            un = wu[ivec * ohf + j, c * k * k + kh * k + kw, :]
                            Wt[row, :, kh, c0:c0 + cout] = un - wp[:, c, kh, kw]
                            Wt[row, :, kh, c1:c1 + cout] = wp[:, c, kh, kw]
                            Wt[row, :, kh, c2:c2 + cout] = wg[c, kh, kw]
                Wt[brow, :, 0, c0:c0 + cout] = bu[:, ivec, j].T - bp
                Wt[brow, :, 0, c1:c1 + cout] = bp
                Wt[brow, :, 0, c2:c2 + cout] = bg
            arrs[f"wb{blk}_{jg}"] = Wt.astype(ml_dtypes.bfloat16)

    wfc = np.zeros((33, 4), np.float32)
    wfc[0:32] = np.asarray(inputs["fc_w"], np.float32)
    wfc[32] = np.asarray(inputs["fc_b"], np.float32)
    arrs["wfc"] = wfc.astype(ml_dtypes.bfloat16)
    arrs["ones"] = np.ones((1, 15 * NB), ml_dtypes.bfloat16)
    return arrs


def make_in_maps(inputs):
    warrs = _prep_weights(inputs)
    x = np.asarray(inputs["x"], np.float32)
    # [w, c, h, b] view of the live x region, rows (w*3+c)
    xt = np.ascontiguousarray(x[:, :, 0:47, 0:47].transpose(3, 1, 2, 0))
    xt = xt.reshape(141, 47, BATCH).astype(ml_dtypes.bfloat16)
    in_maps = []
    for ci in range(N_CORES):
        xc = xt[:, :, ci * NB:(ci + 1) * NB]
        xprep = np.empty((149, 47, NB), ml_dtypes.bfloat16)
        xprep[0:78] = xc[0:78]          # slab0: w 0..25
        xprep[78] = 1.0
        xprep[79:148] = xc[72:141]      # slab1: w 24..46
        xprep[148] = 1.0
        m = {"xprep": xprep}
        m.update(warrs)
        in_maps.append(m)
    return in_maps


def kernel(**inputs):
    if "nc" not in _CACHE:
        _CACHE["nc"] = _build()
    nc = _CACHE["nc"]
    in_maps = make_in_maps(inputs)
    res = run_bass_kernel_spmd(nc, in_maps, core_ids=list(range(N_CORES)))
    out = np.concatenate([res.results[c]["out"].T for c in range(N_CORES)], axis=0)
    return out.astype(np.float32)
